# revision 7
# baseline (speedup 1.0000x reference)
"""Expert-parallel top-2 MoE adapter kernel for 8 Trainium2 NeuronCores.

Problem (hardcoded shapes): B=2, S=2048, H=4096, E=16 experts, top-2, A=512.

reference math:
    logits = x_rtr @ router_w.T                      [T, E]
    probs  = softmax(logits); top-2 renormalized -> w[t, e] (2 nonzeros/row)
    h      = silu(x_in @ w_down[e])                  per (token, expert)
    mix    = sum_e w[t,e] * (h @ w_up[e])
    out    = mix + x_out * sum_e w[t,e]

Only the two selected experts per token contribute (w=0 elsewhere), so we
compute the sparse problem: tokens are grouped by expert on the host (the
routing decides the sharding), experts are paired onto the 8 cores for load
balance, and each core runs a grouped GEMM pipeline:
    down-proj (bf16 matmul, fp32 PSUM) -> SiLU -> x gate weight ->
    PE transpose -> up-proj (bf16 matmul, fp32 PSUM) -> DMA out fp32
The host scatter-adds per-unit outputs and adds the x_out * sum_w term.
"""

import math
from contextlib import ExitStack

import ml_dtypes
import numpy as np

B, S, H = 2, 2048, 4096
E, K_TOP, A = 16, 2, 512
T = B * S
P = 128
KT = H // P          # 32 k-tiles in the down-proj contraction
AT = A // P          # 4 a-tiles in the up-proj contraction
NCHUNK = 512         # matmul free-dim / PSUM bank width (fp32)
HC = H // NCHUNK     # 8 output column chunks
N_CORES = 8
SCALING = 1.0

_BF16 = ml_dtypes.bfloat16

_nc_cache: dict = {}


def build_bass(CA: int, CB: int, repeat: int = 1):
    """Build + finalize the per-core SPMD Bass program for CA + CB token tiles
    (CA tiles use expert weight slot 0, CB tiles use slot 1).

    repeat > 1 re-emits the whole compute body N times (identical results) so
    the harness can measure steady-state device time by slope."""
    key = (CA, CB, repeat)
    if key in _nc_cache:
        return _nc_cache[key]

    import concourse.mybir as mybir
    import concourse.tile as tile
    from concourse import bacc
    from concourse.masks import make_identity

    f32 = mybir.dt.float32
    bf16 = mybir.dt.bfloat16
    U = CA + CB

    nc = bacc.Bacc()
    xt = nc.dram_tensor("xt", [U, H, P], bf16, kind="ExternalInput")
    wd = nc.dram_tensor("wd", [2, H, A], bf16, kind="ExternalInput")
    wu = nc.dram_tensor("wu", [2, A, H], bf16, kind="ExternalInput")
    sc = nc.dram_tensor("sc", [P, U], f32, kind="ExternalInput")
    y = nc.dram_tensor("y", [U, P, H], f32, kind="ExternalOutput")

    with tile.TileContext(nc) as tc:
        with ExitStack() as ctx:
            wpool = ctx.enter_context(tc.tile_pool(name="weights", bufs=1))
            cpool = ctx.enter_context(tc.tile_pool(name="consts", bufs=1))
            xpool = ctx.enter_context(tc.tile_pool(name="xt", bufs=3))
            hpool = ctx.enter_context(tc.tile_pool(name="h", bufs=2))
            hspool = ctx.enter_context(tc.tile_pool(name="hs", bufs=2))
            htpool = ctx.enter_context(tc.tile_pool(name="ht", bufs=2))
            ypool = ctx.enter_context(tc.tile_pool(name="y", bufs=2))
            psh = ctx.enter_context(tc.tile_pool(name="psh", bufs=2, space="PSUM"))
            pst = ctx.enter_context(tc.tile_pool(name="pst", bufs=2, space="PSUM"))
            psy = ctx.enter_context(tc.tile_pool(name="psy", bufs=3, space="PSUM"))

            ident = cpool.tile([P, P], bf16)
            make_identity(nc, ident)
            sc_sb = cpool.tile([P, U], f32)
            nc.sync.dma_start(sc_sb[:], sc[:])

            wd_sb = wpool.tile([P, 2, KT, A], bf16)
            wu_sb = wpool.tile([P, 2, AT, H], bf16)
            # chunked weight loads so the first matmuls can start early
            for s in range(2):
                wd_r = wd[s].rearrange("(ko p) a -> p ko a", p=P)
                for c in range(4):
                    nc.sync.dma_start(
                        wd_sb[:, s, c * (KT // 4):(c + 1) * (KT // 4), :],
                        wd_r[:, c * (KT // 4):(c + 1) * (KT // 4), :],
                    )
                wu_r = wu[s].rearrange("(ao p) h -> p ao h", p=P)
                for a in range(AT):
                    nc.sync.dma_start(wu_sb[:, s, a, :], wu_r[:, a, :])

            for _rep in range(repeat):
              for u in range(U):
                s = 0 if u < CA else 1
                xt_t = xpool.tile([P, KT, P], bf16, tag="xt")
                nc.sync.dma_start(xt_t[:], xt[u].rearrange("(ko p) t -> p ko t", p=P))

                ph = psh.tile([P, A], f32, tag="ph")
                for k in range(KT):
                    nc.tensor.matmul(
                        ph[:],
                        xt_t[:, k:k + 1, :],
                        wd_sb[:, s, k:k + 1, :],
                        start=(k == 0),
                        stop=(k == KT - 1),
                    )

                h = hpool.tile([P, A], f32, tag="h")
                nc.scalar.activation(h[:], ph[:], mybir.ActivationFunctionType.Silu)
                hs = hspool.tile([P, A], bf16, tag="hs")
                nc.vector.tensor_scalar_mul(hs[:], h[:], sc_sb[:, u:u + 1])

                ht = htpool.tile([P, AT, P], bf16, tag="ht")
                for j in range(AT):
                    ptr = pst.tile([P, P], bf16, tag="ptr")
                    nc.tensor.transpose(ptr[:], hs[:, j * P:(j + 1) * P], ident[:])
                    nc.vector.tensor_copy(ht[:, j, :], ptr[:])

                y_sb = ypool.tile([P, H], f32, tag="y")
                for hc in range(HC):
                    py = psy.tile([P, NCHUNK], f32, tag="py")
                    for a in range(AT):
                        nc.tensor.matmul(
                            py[:],
                            ht[:, a:a + 1, :],
                            wu_sb[:, s, a:a + 1, hc * NCHUNK:(hc + 1) * NCHUNK],
                            start=(a == 0),
                            stop=(a == AT - 1),
                        )
                    nc.vector.tensor_copy(y_sb[:, hc * NCHUNK:(hc + 1) * NCHUNK], py[:])
                nc.sync.dma_start(y[u], y_sb[:])

    nc.finalize()
    _nc_cache[key] = nc
    return nc


def route(x_rtr_flat: np.ndarray, router_w: np.ndarray):
    """Host routing: fp32 logits (returned as output 2), fp64 softmax/top-2."""
    logits = x_rtr_flat @ router_w.T  # [T, E] fp32
    lg = logits.astype(np.float64)
    lg -= lg.max(axis=-1, keepdims=True)
    p = np.exp(lg)
    p /= p.sum(axis=-1, keepdims=True)
    ar = np.arange(T)
    i1 = p.argmax(axis=-1)
    pm = p.copy()
    pm[ar, i1] = -1.0
    i2 = pm.argmax(axis=-1)
    p1 = p[ar, i1]
    p2 = p[ar, i2]
    denom = p1 + p2
    w1 = (p1 / denom).astype(np.float32)
    w2 = (p2 / denom).astype(np.float32)
    sumw = w1 + w2
    return logits, i1, i2, w1, w2, sumw


def plan_dispatch(i1, i2, w1, w2):
    """Group tokens by expert, pair experts onto cores, fix tile capacities."""
    toks_e, wts_e = [], []
    for e in range(E):
        m1 = i1 == e
        m2 = i2 == e
        toks = np.concatenate([np.nonzero(m1)[0], np.nonzero(m2)[0]])
        wts = np.concatenate([w1[m1], w2[m2]]).astype(np.float32)
        toks_e.append(toks)
        wts_e.append(wts)
    ntiles = [max(1, math.ceil(len(t) / P)) for t in toks_e]
    order = sorted(range(E), key=lambda e: -ntiles[e])
    pairs = [(order[i], order[2 * N_CORES - 1 - i]) for i in range(N_CORES)]
    CA = max(ntiles[a] for a, _ in pairs)
    CB = max(ntiles[b] for _, b in pairs)
    return toks_e, wts_e, ntiles, pairs, CA, CB


def build_in_maps(x_in_flat, w_down, w_up, toks_e, wts_e, ntiles, pairs, CA, CB):
    U = CA + CB
    x_bf = x_in_flat.astype(_BF16)
    in_maps = []
    unit_tokens = []  # per core: list of (unit_idx, token_ids)
    for eA, eB in pairs:
        xt_host = np.zeros((U, H, P), dtype=_BF16)
        sc_host = np.zeros((P, U), dtype=np.float32)
        units = []
        for s, (e, base, cap) in enumerate([(eA, 0, CA), (eB, CA, CB)]):
            toks, wts = toks_e[e], wts_e[e]
            for t in range(ntiles[e]):
                chunk = toks[t * P:(t + 1) * P]
                if len(chunk) == 0:
                    continue
                u = base + t
                xt_host[u, :, :len(chunk)] = x_bf[chunk].T
                sc_host[:len(chunk), u] = wts[t * P:(t + 1) * P]
                units.append((u, chunk))
        wd_host = np.ascontiguousarray(w_down[[eA, eB]].astype(_BF16))
        wu_host = np.ascontiguousarray(w_up[[eA, eB]].astype(_BF16))
        in_maps.append({"xt": xt_host, "wd": wd_host, "wu": wu_host, "sc": sc_host})
        unit_tokens.append(units)
    return in_maps, unit_tokens


def combine(x_out_flat, sumw, results, unit_tokens):
    out = x_out_flat * sumw[:, None]
    for core, units in enumerate(unit_tokens):
        yc = results[core]["y"]
        for u, chunk in units:
            out[chunk] += yc[u, :len(chunk), :]
    return out


def prepare(inputs):
    """Everything up to (nc, in_maps) + closure data for combine."""
    x_in = np.ascontiguousarray(np.asarray(inputs["input_hidden_states"], np.float32).reshape(T, H))
    x_out = np.ascontiguousarray(np.asarray(inputs["output_hidden_states"], np.float32).reshape(T, H))
    x_rtr = np.ascontiguousarray(np.asarray(inputs["router_hidden_states"], np.float32).reshape(T, H))
    router_w = np.asarray(inputs["router_w"], np.float32)
    w_down = np.asarray(inputs["w_down"], np.float32)
    w_up = np.asarray(inputs["w_up"], np.float32)

    logits, i1, i2, w1, w2, sumw = route(x_rtr, router_w)
    toks_e, wts_e, ntiles, pairs, CA, CB = plan_dispatch(i1, i2, w1, w2)
    global _last_plan
    _last_plan = (CA, CB)
    nc = build_bass(CA, CB)
    in_maps, unit_tokens = build_in_maps(
        x_in, w_down, w_up, toks_e, wts_e, ntiles, pairs, CA, CB
    )
    return nc, in_maps, unit_tokens, x_out, sumw, logits


def kernel(**inputs):
    from concourse.bass_utils import run_bass_kernel_spmd

    nc, in_maps, unit_tokens, x_out, sumw, logits = prepare(inputs)
    res = run_bass_kernel_spmd(nc, in_maps, list(range(N_CORES)))
    out = combine(x_out, sumw, res.results, unit_tokens)
    if SCALING != 1.0:
        out = (out - x_out * sumw[:, None]) * SCALING + x_out * sumw[:, None]
    return out.reshape(B, S, H).astype(np.float32), logits


# revision 14
# speedup vs baseline: 33.0702x; 33.0702x over previous
"""Expert-parallel top-2 MoE adapter kernel for 8 Trainium2 NeuronCores.

Problem (hardcoded shapes): B=2, S=2048, H=4096, E=16 experts, top-2, A=512.

reference math:
    logits = x_rtr @ router_w.T                      [T, E]
    probs  = softmax(logits); top-2 renormalized -> w[t, e] (2 nonzeros/row)
    h      = silu(x_in @ w_down[e])                  per (token, expert)
    mix    = sum_e w[t,e] * (h @ w_up[e])
    out    = mix + x_out * sum_e w[t,e]

Only the two selected experts per token contribute (w=0 elsewhere), so we
compute the sparse problem: tokens are grouped by expert on the host (the
routing decides the sharding), experts are paired onto the 8 cores for load
balance, and each core runs a grouped GEMM pipeline:
    down-proj (bf16 matmul, fp32 PSUM) -> SiLU -> x gate weight ->
    PE transpose -> up-proj (bf16 matmul, fp32 PSUM) -> DMA out fp32
The host scatter-adds per-unit outputs and adds the x_out * sum_w term.
"""

import math
from contextlib import ExitStack

import ml_dtypes
import numpy as np

B, S, H = 2, 2048, 4096
E, K_TOP, A = 16, 2, 512
T = B * S
P = 128
KT = H // P          # 32 k-tiles in the down-proj contraction
AT = A // P          # 4 a-tiles in the up-proj contraction
NCHUNK = 512         # matmul free-dim / PSUM bank width (fp32)
HC = H // NCHUNK     # 8 output column chunks
N_CORES = 8
SCALING = 1.0

_BF16 = ml_dtypes.bfloat16

_nc_cache: dict = {}


def build_bass(CA: int, CB: int, repeat: int = 1, loop_repeat: int | None = None):
    """Build + finalize the per-core SPMD Bass program for CA + CB token tiles
    (CA tiles use expert weight slot 0, CB tiles use slot 1).

    repeat > 1 re-emits the whole compute body N times (identical results) so
    the harness can measure steady-state device time by slope; loop_repeat
    wraps the body in a hardware For_i loop instead (cheap large repeats)."""
    key = (CA, CB, repeat, loop_repeat)
    if key in _nc_cache:
        return _nc_cache[key]

    import concourse.mybir as mybir
    import concourse.tile as tile
    from concourse import bacc
    from concourse.masks import make_identity

    f32 = mybir.dt.float32
    bf16 = mybir.dt.bfloat16
    U = CA + CB

    nc = bacc.Bacc()
    # all inputs host-pre-interleaved so DMAs are contiguous per partition:
    # xt[u, p, ko, t] = x_in[tok[u][t], ko*128 + p]   (transposed token tiles)
    # wd[s, p, ko, a] = w_down[e_s][ko*128 + p, a]
    # wu[s, p, ao, h] = w_up[e_s][ao*128 + p, h]
    xt = nc.dram_tensor("xt", [U, P, KT, P], bf16, kind="ExternalInput")
    wd = nc.dram_tensor("wd", [2, P, KT, A], bf16, kind="ExternalInput")
    wu = nc.dram_tensor("wu", [2, P, AT, H], bf16, kind="ExternalInput")
    sc = nc.dram_tensor("sc", [P, U], f32, kind="ExternalInput")
    y = nc.dram_tensor("y", [U, P, H], f32, kind="ExternalOutput")

    with tile.TileContext(nc) as tc:
        with ExitStack() as ctx:
            wpool = ctx.enter_context(tc.tile_pool(name="weights", bufs=1))
            cpool = ctx.enter_context(tc.tile_pool(name="consts", bufs=1))
            xpool = ctx.enter_context(tc.tile_pool(name="xt", bufs=3))
            hpool = ctx.enter_context(tc.tile_pool(name="h", bufs=2))
            hspool = ctx.enter_context(tc.tile_pool(name="hs", bufs=2))
            htpool = ctx.enter_context(tc.tile_pool(name="ht", bufs=2))
            ypool = ctx.enter_context(tc.tile_pool(name="y", bufs=2))
            psh = ctx.enter_context(tc.tile_pool(name="psh", bufs=2, space="PSUM"))
            pst = ctx.enter_context(tc.tile_pool(name="pst", bufs=2, space="PSUM"))
            psy = ctx.enter_context(tc.tile_pool(name="psy", bufs=3, space="PSUM"))

            ident = cpool.tile([P, P], bf16)
            make_identity(nc, ident)
            sc_sb = cpool.tile([P, U], f32)
            nc.sync.dma_start(sc_sb[:], sc[:])

            wd_sb = wpool.tile([P, 2, KT, A], bf16)
            wu_sb = wpool.tile([P, 2, AT, H], bf16)
            # chunked weight loads so the first matmuls can start early
            for s in range(2):
                for c in range(4):
                    nc.sync.dma_start(
                        wd_sb[:, s, c * (KT // 4):(c + 1) * (KT // 4), :],
                        wd[s, :, c * (KT // 4):(c + 1) * (KT // 4), :],
                    )
                for a in range(AT):
                    nc.sync.dma_start(wu_sb[:, s, a, :], wu[s, :, a, :])

            def emit_unit(u):
                s = 0 if u < CA else 1
                xt_t = xpool.tile([P, KT, P], bf16, tag="xt")
                nc.sync.dma_start(xt_t[:], xt[u])

                ph = psh.tile([P, A], f32, tag="ph")
                for k in range(KT):
                    nc.tensor.matmul(
                        ph[:],
                        xt_t[:, k:k + 1, :],
                        wd_sb[:, s, k:k + 1, :],
                        start=(k == 0),
                        stop=(k == KT - 1),
                    )

                h = hpool.tile([P, A], f32, tag="h")
                nc.scalar.activation(h[:], ph[:], mybir.ActivationFunctionType.Silu)
                hs = hspool.tile([P, A], bf16, tag="hs")
                nc.vector.tensor_scalar_mul(hs[:], h[:], sc_sb[:, u:u + 1])

                ht = htpool.tile([P, AT, P], bf16, tag="ht")
                for j in range(AT):
                    ptr = pst.tile([P, P], bf16, tag="ptr")
                    nc.tensor.transpose(ptr[:], hs[:, j * P:(j + 1) * P], ident[:])
                    nc.vector.tensor_copy(ht[:, j, :], ptr[:])

                y_sb = ypool.tile([P, H], f32, tag="y")
                for hc in range(HC):
                    py = psy.tile([P, NCHUNK], f32, tag="py")
                    for a in range(AT):
                        nc.tensor.matmul(
                            py[:],
                            ht[:, a:a + 1, :],
                            wu_sb[:, s, a:a + 1, hc * NCHUNK:(hc + 1) * NCHUNK],
                            start=(a == 0),
                            stop=(a == AT - 1),
                        )
                    nc.vector.tensor_copy(
                        y_sb[:, hc * NCHUNK:(hc + 1) * NCHUNK], py[:]
                    )
                nc.sync.dma_start(y[u], y_sb[:])

            def emit_body():
                for _rep in range(repeat):
                    for u in range(U):
                        emit_unit(u)

            if loop_repeat is not None:
                with tc.For_i(0, loop_repeat):
                    emit_body()
            else:
                emit_body()

    nc.finalize()
    _nc_cache[key] = nc
    return nc


def route(x_rtr_flat: np.ndarray, router_w: np.ndarray):
    """Host routing: fp32 logits (returned as output 2), fp64 softmax/top-2."""
    logits = x_rtr_flat @ router_w.T  # [T, E] fp32
    lg = logits.astype(np.float64)
    lg -= lg.max(axis=-1, keepdims=True)
    p = np.exp(lg)
    p /= p.sum(axis=-1, keepdims=True)
    ar = np.arange(T)
    i1 = p.argmax(axis=-1)
    pm = p.copy()
    pm[ar, i1] = -1.0
    i2 = pm.argmax(axis=-1)
    p1 = p[ar, i1]
    p2 = p[ar, i2]
    denom = p1 + p2
    w1 = (p1 / denom).astype(np.float32)
    w2 = (p2 / denom).astype(np.float32)
    sumw = w1 + w2
    return logits, i1, i2, w1, w2, sumw


def plan_dispatch(i1, i2, w1, w2):
    """Group tokens by expert, pair experts onto cores, fix tile capacities."""
    toks_e, wts_e = [], []
    for e in range(E):
        m1 = i1 == e
        m2 = i2 == e
        toks = np.concatenate([np.nonzero(m1)[0], np.nonzero(m2)[0]])
        wts = np.concatenate([w1[m1], w2[m2]]).astype(np.float32)
        toks_e.append(toks)
        wts_e.append(wts)
    ntiles = [max(1, math.ceil(len(t) / P)) for t in toks_e]
    order = sorted(range(E), key=lambda e: -ntiles[e])
    pairs = [(order[i], order[2 * N_CORES - 1 - i]) for i in range(N_CORES)]
    CA = max(ntiles[a] for a, _ in pairs)
    CB = max(ntiles[b] for _, b in pairs)
    return toks_e, wts_e, ntiles, pairs, CA, CB


def build_in_maps(x_in_flat, w_down, w_up, toks_e, wts_e, ntiles, pairs, CA, CB):
    U = CA + CB
    x_bf = x_in_flat.astype(_BF16)
    in_maps = []
    unit_tokens = []  # per core: list of (unit_idx, token_ids)
    for eA, eB in pairs:
        xt_host = np.zeros((U, P, KT, P), dtype=_BF16)
        sc_host = np.zeros((P, U), dtype=np.float32)
        units = []
        for s, (e, base, cap) in enumerate([(eA, 0, CA), (eB, CA, CB)]):
            toks, wts = toks_e[e], wts_e[e]
            for t in range(ntiles[e]):
                chunk = toks[t * P:(t + 1) * P]
                if len(chunk) == 0:
                    continue
                u = base + t
                # [n, H] -> [H, n] -> [KT, P, n] -> [P, KT, n]
                xt_host[u, :, :, :len(chunk)] = (
                    x_bf[chunk].T.reshape(KT, P, len(chunk)).transpose(1, 0, 2)
                )
                sc_host[:len(chunk), u] = wts[t * P:(t + 1) * P]
                units.append((u, chunk))
        wd_host = np.ascontiguousarray(
            w_down[[eA, eB]].astype(_BF16).reshape(2, KT, P, A).transpose(0, 2, 1, 3)
        )
        wu_host = np.ascontiguousarray(
            w_up[[eA, eB]].astype(_BF16).reshape(2, AT, P, H).transpose(0, 2, 1, 3)
        )
        in_maps.append({"xt": xt_host, "wd": wd_host, "wu": wu_host, "sc": sc_host})
        unit_tokens.append(units)
    return in_maps, unit_tokens


def combine(x_out_flat, sumw, results, unit_tokens):
    out = x_out_flat * sumw[:, None]
    for core, units in enumerate(unit_tokens):
        yc = results[core]["y"]
        for u, chunk in units:
            out[chunk] += yc[u, :len(chunk), :]
    return out


def prepare(inputs):
    """Everything up to (nc, in_maps) + closure data for combine."""
    x_in = np.ascontiguousarray(np.asarray(inputs["input_hidden_states"], np.float32).reshape(T, H))
    x_out = np.ascontiguousarray(np.asarray(inputs["output_hidden_states"], np.float32).reshape(T, H))
    x_rtr = np.ascontiguousarray(np.asarray(inputs["router_hidden_states"], np.float32).reshape(T, H))
    router_w = np.asarray(inputs["router_w"], np.float32)
    w_down = np.asarray(inputs["w_down"], np.float32)
    w_up = np.asarray(inputs["w_up"], np.float32)

    logits, i1, i2, w1, w2, sumw = route(x_rtr, router_w)
    toks_e, wts_e, ntiles, pairs, CA, CB = plan_dispatch(i1, i2, w1, w2)
    global _last_plan
    _last_plan = (CA, CB)
    nc = build_bass(CA, CB)
    in_maps, unit_tokens = build_in_maps(
        x_in, w_down, w_up, toks_e, wts_e, ntiles, pairs, CA, CB
    )
    return nc, in_maps, unit_tokens, x_out, sumw, logits


def kernel(**inputs):
    from concourse.bass_utils import run_bass_kernel_spmd

    nc, in_maps, unit_tokens, x_out, sumw, logits = prepare(inputs)
    res = run_bass_kernel_spmd(nc, in_maps, list(range(N_CORES)))
    out = combine(x_out, sumw, res.results, unit_tokens)
    if SCALING != 1.0:
        out = (out - x_out * sumw[:, None]) * SCALING + x_out * sumw[:, None]
    return out.reshape(B, S, H).astype(np.float32), logits


# revision 26
# speedup vs baseline: 61.5498x; 1.8612x over previous
"""Expert-parallel top-2 MoE adapter kernel for 8 Trainium2 NeuronCores.

Problem (hardcoded shapes): B=2, S=2048, H=4096, E=16 experts, top-2, A=512.

reference math:
    logits = x_rtr @ router_w.T                      [T, E]
    probs  = softmax(logits); top-2 renormalized -> w[t, e] (2 nonzeros/row)
    h      = silu(x_in @ w_down[e])                  per (token, expert)
    mix    = sum_e w[t,e] * (h @ w_up[e])
    out    = mix + x_out * sum_e w[t,e]

Only the two selected experts per token contribute (w=0 elsewhere), so we
compute the sparse problem: tokens are grouped by expert on the host (the
routing decides the sharding), experts are paired onto the 8 cores for load
balance, and each core runs a grouped GEMM pipeline:
    down-proj (bf16 matmul, fp32 PSUM) -> SiLU -> x gate weight ->
    PE transpose -> up-proj (bf16 matmul, fp32 PSUM) -> DMA out fp32
The host scatter-adds per-unit outputs and adds the x_out * sum_w term.
"""

import math
from contextlib import ExitStack

import ml_dtypes
import numpy as np

B, S, H = 2, 2048, 4096
E, K_TOP, A = 16, 2, 512
T = B * S
P = 128
KT = H // P          # 32 k-tiles in the down-proj contraction
AT = A // P          # 4 a-tiles in the up-proj contraction
NCHUNK = 512         # matmul free-dim / PSUM bank width (fp32)
HC = H // NCHUNK     # 8 output column chunks
N_CORES = 8
SCALING = 1.0

_BF16 = ml_dtypes.bfloat16

_nc_cache: dict = {}


def build_bass(
    CA: int,
    CB: int,
    repeat: int = 1,
    loop_repeat: int | None = None,
    probe: str = "full",
    w_eng: str = "gpsimd",
    roll: bool = True,
):
    """Build + finalize the per-core SPMD Bass program for CA + CB token tiles
    (CA tiles use expert weight slot 0, CB tiles use slot 1).

    repeat > 1 re-emits the whole compute body N times (identical results) so
    the harness can measure steady-state device time by slope; loop_repeat
    wraps the body in a hardware For_i loop instead (cheap large repeats).
    probe selects timing-probe variants (wrong math, used only to attribute
    time): "down" = down-proj+silu only; "noup" = no up-proj matmuls;
    "notr" = skip transposes (garbage lhsT for up); "nodma" = no y DMA."""
    key = (CA, CB, repeat, loop_repeat, probe, w_eng, roll)
    if key in _nc_cache:
        return _nc_cache[key]

    import concourse.mybir as mybir
    import concourse.tile as tile
    from concourse import bacc
    from concourse.masks import make_identity

    f32 = mybir.dt.float32
    bf16 = mybir.dt.bfloat16
    U = CA + CB

    nc = bacc.Bacc()
    # all inputs host-pre-interleaved so DMAs are contiguous per partition:
    # xt[u, p, ko, t] = x_in[tok[u][t], ko*128 + p]   (transposed token tiles)
    # wd[s, p, ko, a] = w_down[e_s][ko*128 + p, a]
    # wu[s, p, ao, h] = w_up[e_s][ao*128 + p, h]
    xt = nc.dram_tensor("xt", [U, P, KT, P], bf16, kind="ExternalInput")
    wd = nc.dram_tensor("wd", [2, P, KT, A], bf16, kind="ExternalInput")
    wu = nc.dram_tensor("wu", [2, P, AT, H], bf16, kind="ExternalInput")
    sc = nc.dram_tensor("sc", [P, U], f32, kind="ExternalInput")
    y = nc.dram_tensor("y", [U, P, H], f32, kind="ExternalOutput")

    with tile.TileContext(nc) as tc:
        with ExitStack() as ctx:
            wpool = ctx.enter_context(tc.tile_pool(name="weights", bufs=1))
            cpool = ctx.enter_context(tc.tile_pool(name="consts", bufs=1))
            xpool = ctx.enter_context(tc.tile_pool(name="xt", bufs=3))
            hpool = ctx.enter_context(tc.tile_pool(name="h", bufs=2))
            hspool = ctx.enter_context(tc.tile_pool(name="hs", bufs=2))
            htpool = ctx.enter_context(tc.tile_pool(name="ht", bufs=2))
            ypool = ctx.enter_context(tc.tile_pool(name="y", bufs=2))
            psh = ctx.enter_context(tc.tile_pool(name="psh", bufs=2, space="PSUM"))
            pst = ctx.enter_context(tc.tile_pool(name="pst", bufs=2, space="PSUM"))
            psy = ctx.enter_context(tc.tile_pool(name="psy", bufs=3, space="PSUM"))

            ident = cpool.tile([P, P], bf16)
            make_identity(nc, ident)
            sc_sb = cpool.tile([P, U], f32)
            getattr(nc, w_eng).dma_start(sc_sb[:], sc[:])

            wd_sb = wpool.tile([P, 2, KT, A], bf16)
            wu_sb = wpool.tile([P, 2, AT, H], bf16)
            prefetched: dict = {}

            def fetch_xt(u):
                xt_t = xpool.tile([P, KT, P], bf16, tag="xt", name="xt_t")
                nc.sync.dma_start(xt_t[:], xt[u])
                return xt_t

            # first x tiles + chunked weight loads, ordered so the first
            # matmuls can start as early as possible (SP issues DMAs in order)
            if loop_repeat is None:
                for u in range(min(3, U)):
                    prefetched[u] = fetch_xt(u)
            for s in range(2):
                for c in range(4):
                    getattr(nc, w_eng).dma_start(
                        wd_sb[:, s, c * (KT // 4):(c + 1) * (KT // 4), :],
                        wd[s, :, c * (KT // 4):(c + 1) * (KT // 4), :],
                    )
                for a in range(AT):
                    getattr(nc, w_eng).dma_start(wu_sb[:, s, a, :], wu[s, :, a, :])

            def emit_unit(u):
                s = 0 if u < CA else 1
                xt_t = prefetched.pop(u, None)
                if xt_t is None:
                    xt_t = fetch_xt(u)
                # roll the prefetch window forward BEFORE emitting this unit's
                # y store: the SP FIFO issues in order, and y's sem wait would
                # otherwise block the next xt loads until this unit finishes
                if roll:
                    for nxt in (u + 1, u + 2):
                        if nxt < U and nxt not in prefetched:
                            prefetched[nxt] = fetch_xt(nxt)

                ph = psh.tile([P, A], f32, tag="ph")
                for k in range(KT):
                    nc.tensor.matmul(
                        ph[:],
                        xt_t[:, k:k + 1, :],
                        wd_sb[:, s, k:k + 1, :],
                        start=(k == 0),
                        stop=(k == KT - 1),
                    )

                h = hpool.tile([P, A], f32, tag="h")
                nc.scalar.activation(h[:], ph[:], mybir.ActivationFunctionType.Silu)
                hs = hspool.tile([P, A], bf16, tag="hs")
                nc.vector.tensor_scalar_mul(hs[:], h[:], sc_sb[:, u:u + 1])
                if probe == "down":
                    nc.sync.dma_start(y[u, :, :A], h[:])
                    return

                ht = htpool.tile([P, AT, P], bf16, tag="ht")
                if probe == "notr":
                    nc.vector.tensor_copy(ht[:], hs[:].rearrange("p (a q) -> p a q", a=AT))
                else:
                    for j in range(AT):
                        ptr = pst.tile([P, P], bf16, tag="ptr")
                        nc.tensor.transpose(ptr[:], hs[:, j * P:(j + 1) * P], ident[:])
                        nc.vector.tensor_copy(ht[:, j, :], ptr[:])
                if probe == "noup":
                    nc.sync.dma_start(y[u, :, :A], h[:])
                    return

                y_sb = ypool.tile([P, H], f32, tag="y")
                for hc in range(HC):
                    py = psy.tile([P, NCHUNK], f32, tag="py")
                    for a in range(AT):
                        nc.tensor.matmul(
                            py[:],
                            ht[:, a:a + 1, :],
                            wu_sb[:, s, a:a + 1, hc * NCHUNK:(hc + 1) * NCHUNK],
                            start=(a == 0),
                            stop=(a == AT - 1),
                        )
                    nc.vector.tensor_copy(
                        y_sb[:, hc * NCHUNK:(hc + 1) * NCHUNK], py[:]
                    )
                if probe == "nodma":
                    return
                nc.sync.dma_start(y[u], y_sb[:])

            def emit_body():
                for _rep in range(repeat):
                    for u in range(U):
                        emit_unit(u)

            if loop_repeat is not None:
                with tc.For_i(0, loop_repeat):
                    emit_body()
            else:
                emit_body()

    nc.finalize()
    _nc_cache[key] = nc
    return nc


def route(x_rtr_flat: np.ndarray, router_w: np.ndarray):
    """Host routing: fp32 logits (returned as output 2), fp64 softmax/top-2."""
    logits = x_rtr_flat @ router_w.T  # [T, E] fp32
    lg = logits.astype(np.float64)
    lg -= lg.max(axis=-1, keepdims=True)
    p = np.exp(lg)
    p /= p.sum(axis=-1, keepdims=True)
    ar = np.arange(T)
    i1 = p.argmax(axis=-1)
    pm = p.copy()
    pm[ar, i1] = -1.0
    i2 = pm.argmax(axis=-1)
    p1 = p[ar, i1]
    p2 = p[ar, i2]
    denom = p1 + p2
    w1 = (p1 / denom).astype(np.float32)
    w2 = (p2 / denom).astype(np.float32)
    sumw = w1 + w2
    return logits, i1, i2, w1, w2, sumw


def plan_dispatch(i1, i2, w1, w2):
    """Group tokens by expert, pair experts onto cores, fix tile capacities."""
    toks_e, wts_e = [], []
    for e in range(E):
        m1 = i1 == e
        m2 = i2 == e
        toks = np.concatenate([np.nonzero(m1)[0], np.nonzero(m2)[0]])
        wts = np.concatenate([w1[m1], w2[m2]]).astype(np.float32)
        toks_e.append(toks)
        wts_e.append(wts)
    ntiles = [max(1, math.ceil(len(t) / P)) for t in toks_e]
    order = sorted(range(E), key=lambda e: -ntiles[e])
    pairs = [(order[i], order[2 * N_CORES - 1 - i]) for i in range(N_CORES)]
    CA = max(ntiles[a] for a, _ in pairs)
    CB = max(ntiles[b] for _, b in pairs)
    return toks_e, wts_e, ntiles, pairs, CA, CB


def build_in_maps(x_in_flat, w_down, w_up, toks_e, wts_e, ntiles, pairs, CA, CB):
    U = CA + CB
    x_bf = x_in_flat.astype(_BF16)
    in_maps = []
    unit_tokens = []  # per core: list of (unit_idx, token_ids)
    for eA, eB in pairs:
        xt_host = np.zeros((U, P, KT, P), dtype=_BF16)
        sc_host = np.zeros((P, U), dtype=np.float32)
        units = []
        for s, (e, base, cap) in enumerate([(eA, 0, CA), (eB, CA, CB)]):
            toks, wts = toks_e[e], wts_e[e]
            for t in range(ntiles[e]):
                chunk = toks[t * P:(t + 1) * P]
                if len(chunk) == 0:
                    continue
                u = base + t
                # [n, H] -> [H, n] -> [KT, P, n] -> [P, KT, n]
                xt_host[u, :, :, :len(chunk)] = (
                    x_bf[chunk].T.reshape(KT, P, len(chunk)).transpose(1, 0, 2)
                )
                sc_host[:len(chunk), u] = wts[t * P:(t + 1) * P]
                units.append((u, chunk))
        wd_host = np.ascontiguousarray(
            w_down[[eA, eB]].astype(_BF16).reshape(2, KT, P, A).transpose(0, 2, 1, 3)
        )
        wu_host = np.ascontiguousarray(
            w_up[[eA, eB]].astype(_BF16).reshape(2, AT, P, H).transpose(0, 2, 1, 3)
        )
        in_maps.append({"xt": xt_host, "wd": wd_host, "wu": wu_host, "sc": sc_host})
        unit_tokens.append(units)
    return in_maps, unit_tokens


def combine(x_out_flat, sumw, results, unit_tokens):
    out = x_out_flat * sumw[:, None]
    for core, units in enumerate(unit_tokens):
        yc = results[core]["y"]
        for u, chunk in units:
            out[chunk] += yc[u, :len(chunk), :]
    return out


def prepare(inputs):
    """Everything up to (nc, in_maps) + closure data for combine."""
    x_in = np.ascontiguousarray(np.asarray(inputs["input_hidden_states"], np.float32).reshape(T, H))
    x_out = np.ascontiguousarray(np.asarray(inputs["output_hidden_states"], np.float32).reshape(T, H))
    x_rtr = np.ascontiguousarray(np.asarray(inputs["router_hidden_states"], np.float32).reshape(T, H))
    router_w = np.asarray(inputs["router_w"], np.float32)
    w_down = np.asarray(inputs["w_down"], np.float32)
    w_up = np.asarray(inputs["w_up"], np.float32)

    logits, i1, i2, w1, w2, sumw = route(x_rtr, router_w)
    toks_e, wts_e, ntiles, pairs, CA, CB = plan_dispatch(i1, i2, w1, w2)
    global _last_plan
    _last_plan = (CA, CB)
    nc = build_bass(CA, CB)
    in_maps, unit_tokens = build_in_maps(
        x_in, w_down, w_up, toks_e, wts_e, ntiles, pairs, CA, CB
    )
    return nc, in_maps, unit_tokens, x_out, sumw, logits


def kernel(**inputs):
    from concourse.bass_utils import run_bass_kernel_spmd

    nc, in_maps, unit_tokens, x_out, sumw, logits = prepare(inputs)
    res = run_bass_kernel_spmd(nc, in_maps, list(range(N_CORES)))
    out = combine(x_out, sumw, res.results, unit_tokens)
    if SCALING != 1.0:
        out = (out - x_out * sumw[:, None]) * SCALING + x_out * sumw[:, None]
    return out.reshape(B, S, H).astype(np.float32), logits


# revision 28
# speedup vs baseline: 66.8661x; 1.0864x over previous
"""Expert-parallel top-2 MoE adapter kernel for 8 Trainium2 NeuronCores.

Problem (hardcoded shapes): B=2, S=2048, H=4096, E=16 experts, top-2, A=512.

reference math:
    logits = x_rtr @ router_w.T                      [T, E]
    probs  = softmax(logits); top-2 renormalized -> w[t, e] (2 nonzeros/row)
    h      = silu(x_in @ w_down[e])                  per (token, expert)
    mix    = sum_e w[t,e] * (h @ w_up[e])
    out    = mix + x_out * sum_e w[t,e]

Only the two selected experts per token contribute (w=0 elsewhere), so we
compute the sparse problem: tokens are grouped by expert on the host (the
routing decides the sharding), experts are paired onto the 8 cores for load
balance, and each core runs a grouped GEMM pipeline:
    down-proj (bf16 matmul, fp32 PSUM) -> SiLU -> x gate weight ->
    PE transpose -> up-proj (bf16 matmul, fp32 PSUM) -> DMA out fp32
The host scatter-adds per-unit outputs and adds the x_out * sum_w term.
"""

import math
from contextlib import ExitStack

import ml_dtypes
import numpy as np

B, S, H = 2, 2048, 4096
E, K_TOP, A = 16, 2, 512
T = B * S
P = 128
KT = H // P          # 32 k-tiles in the down-proj contraction
AT = A // P          # 4 a-tiles in the up-proj contraction
NCHUNK = 512         # matmul free-dim / PSUM bank width (fp32)
HC = H // NCHUNK     # 8 output column chunks
N_CORES = 8
SCALING = 1.0

_BF16 = ml_dtypes.bfloat16

_nc_cache: dict = {}


def build_bass(
    CA: int,
    CB: int,
    repeat: int = 1,
    loop_repeat: int | None = None,
    probe: str = "full",
    w_eng: str = "gpsimd",
    roll: bool = True,
    psy_bufs: int = 4,
):
    """Build + finalize the per-core SPMD Bass program for CA + CB token tiles
    (CA tiles use expert weight slot 0, CB tiles use slot 1).

    repeat > 1 re-emits the whole compute body N times (identical results) so
    the harness can measure steady-state device time by slope; loop_repeat
    wraps the body in a hardware For_i loop instead (cheap large repeats).
    probe selects timing-probe variants (wrong math, used only to attribute
    time): "down" = down-proj+silu only; "noup" = no up-proj matmuls;
    "notr" = skip transposes (garbage lhsT for up); "nodma" = no y DMA."""
    key = (CA, CB, repeat, loop_repeat, probe, w_eng, roll, psy_bufs)
    if key in _nc_cache:
        return _nc_cache[key]

    import concourse.mybir as mybir
    import concourse.tile as tile
    from concourse import bacc
    from concourse.masks import make_identity

    f32 = mybir.dt.float32
    bf16 = mybir.dt.bfloat16
    U = CA + CB

    nc = bacc.Bacc()
    # all inputs host-pre-interleaved so DMAs are contiguous per partition:
    # xt[u, p, ko, t] = x_in[tok[u][t], ko*128 + p]   (transposed token tiles)
    # wd[s, p, ko, a] = w_down[e_s][ko*128 + p, a]
    # wu[s, p, ao, h] = w_up[e_s][ao*128 + p, h]
    xt = nc.dram_tensor("xt", [U, P, KT, P], bf16, kind="ExternalInput")
    wd = nc.dram_tensor("wd", [2, P, KT, A], bf16, kind="ExternalInput")
    wu = nc.dram_tensor("wu", [2, P, AT, H], bf16, kind="ExternalInput")
    sc = nc.dram_tensor("sc", [P, U], f32, kind="ExternalInput")
    y = nc.dram_tensor("y", [U, P, H], f32, kind="ExternalOutput")

    with tile.TileContext(nc) as tc:
        with ExitStack() as ctx:
            wpool = ctx.enter_context(tc.tile_pool(name="weights", bufs=1))
            cpool = ctx.enter_context(tc.tile_pool(name="consts", bufs=1))
            xpool = ctx.enter_context(tc.tile_pool(name="xt", bufs=3))
            hpool = ctx.enter_context(tc.tile_pool(name="h", bufs=2))
            hspool = ctx.enter_context(tc.tile_pool(name="hs", bufs=2))
            htpool = ctx.enter_context(tc.tile_pool(name="ht", bufs=2))
            ypool = ctx.enter_context(tc.tile_pool(name="y", bufs=2))
            psh = ctx.enter_context(tc.tile_pool(name="psh", bufs=2, space="PSUM"))
            pst = ctx.enter_context(tc.tile_pool(name="pst", bufs=2, space="PSUM"))
            psy = ctx.enter_context(tc.tile_pool(name="psy", bufs=psy_bufs, space="PSUM"))

            ident = cpool.tile([P, P], bf16)
            make_identity(nc, ident)
            sc_sb = cpool.tile([P, U], f32)
            getattr(nc, w_eng).dma_start(sc_sb[:], sc[:])

            wd_sb = wpool.tile([P, 2, KT, A], bf16)
            wu_sb = wpool.tile([P, 2, AT, H], bf16)
            prefetched: dict = {}

            def fetch_xt(u):
                xt_t = xpool.tile([P, KT, P], bf16, tag="xt", name="xt_t")
                nc.sync.dma_start(xt_t[:], xt[u])
                return xt_t

            # first x tiles + chunked weight loads, ordered so the first
            # matmuls can start as early as possible (SP issues DMAs in order)
            if loop_repeat is None:
                for u in range(min(3, U)):
                    prefetched[u] = fetch_xt(u)
            for s in range(2):
                for c in range(4):
                    getattr(nc, w_eng).dma_start(
                        wd_sb[:, s, c * (KT // 4):(c + 1) * (KT // 4), :],
                        wd[s, :, c * (KT // 4):(c + 1) * (KT // 4), :],
                    )
                for a in range(AT):
                    getattr(nc, w_eng).dma_start(wu_sb[:, s, a, :], wu[s, :, a, :])

            def emit_unit(u):
                s = 0 if u < CA else 1
                xt_t = prefetched.pop(u, None)
                if xt_t is None:
                    xt_t = fetch_xt(u)
                # roll the prefetch window forward BEFORE emitting this unit's
                # y store: the SP FIFO issues in order, and y's sem wait would
                # otherwise block the next xt loads until this unit finishes
                if roll:
                    for nxt in (u + 1, u + 2):
                        if nxt < U and nxt not in prefetched:
                            prefetched[nxt] = fetch_xt(nxt)

                ph = psh.tile([P, A], f32, tag="ph")
                for k in range(KT):
                    nc.tensor.matmul(
                        ph[:],
                        xt_t[:, k:k + 1, :],
                        wd_sb[:, s, k:k + 1, :],
                        start=(k == 0),
                        stop=(k == KT - 1),
                    )

                h = hpool.tile([P, A], f32, tag="h")
                nc.scalar.activation(h[:], ph[:], mybir.ActivationFunctionType.Silu)
                hs = hspool.tile([P, A], bf16, tag="hs")
                nc.vector.tensor_scalar_mul(hs[:], h[:], sc_sb[:, u:u + 1])
                if probe == "down":
                    nc.sync.dma_start(y[u, :, :A], h[:])
                    return

                ht = htpool.tile([P, AT, P], bf16, tag="ht")
                if probe == "notr":
                    nc.vector.tensor_copy(ht[:], hs[:].rearrange("p (a q) -> p a q", a=AT))
                else:
                    for j in range(AT):
                        ptr = pst.tile([P, P], bf16, tag="ptr")
                        nc.tensor.transpose(ptr[:], hs[:, j * P:(j + 1) * P], ident[:])
                        nc.vector.tensor_copy(ht[:, j, :], ptr[:])
                if probe == "noup":
                    nc.sync.dma_start(y[u, :, :A], h[:])
                    return

                y_sb = ypool.tile([P, H], f32, tag="y")
                for hc in range(HC):
                    py = psy.tile([P, NCHUNK], f32, tag="py")
                    for a in range(AT):
                        nc.tensor.matmul(
                            py[:],
                            ht[:, a:a + 1, :],
                            wu_sb[:, s, a:a + 1, hc * NCHUNK:(hc + 1) * NCHUNK],
                            start=(a == 0),
                            stop=(a == AT - 1),
                        )
                    nc.vector.tensor_copy(
                        y_sb[:, hc * NCHUNK:(hc + 1) * NCHUNK], py[:]
                    )
                if probe == "nodma":
                    return
                nc.sync.dma_start(y[u], y_sb[:])

            def emit_body():
                for _rep in range(repeat):
                    for u in range(U):
                        emit_unit(u)

            if loop_repeat is not None:
                with tc.For_i(0, loop_repeat):
                    emit_body()
            else:
                emit_body()

    nc.finalize()
    _nc_cache[key] = nc
    return nc


def route(x_rtr_flat: np.ndarray, router_w: np.ndarray):
    """Host routing: fp32 logits (returned as output 2), fp64 softmax/top-2."""
    logits = x_rtr_flat @ router_w.T  # [T, E] fp32
    lg = logits.astype(np.float64)
    lg -= lg.max(axis=-1, keepdims=True)
    p = np.exp(lg)
    p /= p.sum(axis=-1, keepdims=True)
    ar = np.arange(T)
    i1 = p.argmax(axis=-1)
    pm = p.copy()
    pm[ar, i1] = -1.0
    i2 = pm.argmax(axis=-1)
    p1 = p[ar, i1]
    p2 = p[ar, i2]
    denom = p1 + p2
    w1 = (p1 / denom).astype(np.float32)
    w2 = (p2 / denom).astype(np.float32)
    sumw = w1 + w2
    return logits, i1, i2, w1, w2, sumw


def plan_dispatch(i1, i2, w1, w2):
    """Group tokens by expert, pair experts onto cores, fix tile capacities."""
    toks_e, wts_e = [], []
    for e in range(E):
        m1 = i1 == e
        m2 = i2 == e
        toks = np.concatenate([np.nonzero(m1)[0], np.nonzero(m2)[0]])
        wts = np.concatenate([w1[m1], w2[m2]]).astype(np.float32)
        toks_e.append(toks)
        wts_e.append(wts)
    ntiles = [max(1, math.ceil(len(t) / P)) for t in toks_e]
    order = sorted(range(E), key=lambda e: -ntiles[e])
    pairs = [(order[i], order[2 * N_CORES - 1 - i]) for i in range(N_CORES)]
    CA = max(ntiles[a] for a, _ in pairs)
    CB = max(ntiles[b] for _, b in pairs)
    return toks_e, wts_e, ntiles, pairs, CA, CB


def build_in_maps(x_in_flat, w_down, w_up, toks_e, wts_e, ntiles, pairs, CA, CB):
    U = CA + CB
    x_bf = x_in_flat.astype(_BF16)
    in_maps = []
    unit_tokens = []  # per core: list of (unit_idx, token_ids)
    for eA, eB in pairs:
        xt_host = np.zeros((U, P, KT, P), dtype=_BF16)
        sc_host = np.zeros((P, U), dtype=np.float32)
        units = []
        for s, (e, base, cap) in enumerate([(eA, 0, CA), (eB, CA, CB)]):
            toks, wts = toks_e[e], wts_e[e]
            for t in range(ntiles[e]):
                chunk = toks[t * P:(t + 1) * P]
                if len(chunk) == 0:
                    continue
                u = base + t
                # [n, H] -> [H, n] -> [KT, P, n] -> [P, KT, n]
                xt_host[u, :, :, :len(chunk)] = (
                    x_bf[chunk].T.reshape(KT, P, len(chunk)).transpose(1, 0, 2)
                )
                sc_host[:len(chunk), u] = wts[t * P:(t + 1) * P]
                units.append((u, chunk))
        wd_host = np.ascontiguousarray(
            w_down[[eA, eB]].astype(_BF16).reshape(2, KT, P, A).transpose(0, 2, 1, 3)
        )
        wu_host = np.ascontiguousarray(
            w_up[[eA, eB]].astype(_BF16).reshape(2, AT, P, H).transpose(0, 2, 1, 3)
        )
        in_maps.append({"xt": xt_host, "wd": wd_host, "wu": wu_host, "sc": sc_host})
        unit_tokens.append(units)
    return in_maps, unit_tokens


def combine(x_out_flat, sumw, results, unit_tokens):
    out = x_out_flat * sumw[:, None]
    for core, units in enumerate(unit_tokens):
        yc = results[core]["y"]
        for u, chunk in units:
            out[chunk] += yc[u, :len(chunk), :]
    return out


def prepare(inputs):
    """Everything up to (nc, in_maps) + closure data for combine."""
    x_in = np.ascontiguousarray(np.asarray(inputs["input_hidden_states"], np.float32).reshape(T, H))
    x_out = np.ascontiguousarray(np.asarray(inputs["output_hidden_states"], np.float32).reshape(T, H))
    x_rtr = np.ascontiguousarray(np.asarray(inputs["router_hidden_states"], np.float32).reshape(T, H))
    router_w = np.asarray(inputs["router_w"], np.float32)
    w_down = np.asarray(inputs["w_down"], np.float32)
    w_up = np.asarray(inputs["w_up"], np.float32)

    logits, i1, i2, w1, w2, sumw = route(x_rtr, router_w)
    toks_e, wts_e, ntiles, pairs, CA, CB = plan_dispatch(i1, i2, w1, w2)
    global _last_plan
    _last_plan = (CA, CB)
    nc = build_bass(CA, CB)
    in_maps, unit_tokens = build_in_maps(
        x_in, w_down, w_up, toks_e, wts_e, ntiles, pairs, CA, CB
    )
    return nc, in_maps, unit_tokens, x_out, sumw, logits


def kernel(**inputs):
    from concourse.bass_utils import run_bass_kernel_spmd

    nc, in_maps, unit_tokens, x_out, sumw, logits = prepare(inputs)
    res = run_bass_kernel_spmd(nc, in_maps, list(range(N_CORES)))
    out = combine(x_out, sumw, res.results, unit_tokens)
    if SCALING != 1.0:
        out = (out - x_out * sumw[:, None]) * SCALING + x_out * sumw[:, None]
    return out.reshape(B, S, H).astype(np.float32), logits
